# revision 10
# baseline (speedup 1.0000x reference)
"""Tensor-parallel GQA attention kernel for 8 Trainium2 NeuronCores.

Problem: x[2,2048,2048] -> Attention(16 q heads, 4 kv heads, rotary,
causal) -> out[2,2048,2048].

Sharding: core c handles batch b=c//4 and kv-group g=c%4 (4 q-heads +
1 kv-head). Each core computes its heads' attention output and a
partial O-projection [DIM, S] (output-dim major, fp16); the host sums
the 4 partials per batch and transposes.

v4 design (pure fp16 matmuls, fp32 PSUM):
  - x pre-transposed on host (xT [DIM,S] fp16); contiguous DMA loads,
    chunk-0 split per feature-tile across queues in consumption order.
    Warmup matmuls on a memset tile bridge the ~10us DMA-ring startup
    and lift the HAM clock gate.
  - Projections fi-outer per 512-token chunk with 6 concurrently open
    PSUM groups (V,K,Q0..Q3); V transposed via PE into a dedicated
    fp16 PSUM bank (4 transposes, then 4 evac copies - no same-bank
    PE-write/DVE-read interleave).
  - Attention blocks interleave between projection chunks (attn(qc)
    right after chunk qc+1) so cross-phase queue backlogs hide under
    matmul streams. One unified 8-bank PSUM map serves both phases:
    P1 2x[128,1024]f32 (chunk V|K + Q0|Q1 / attn score pairs + bcast),
    P2 2x[128,512]f32 (chunk Q2,Q3 / attn ones-rows + O-proj tiles),
    P3 1x[128,512]f32 (warmup / attn AV accumulator),
    P4 1x[128,1024]f16 (V transposes).
  - Attention per 512-q chunk: flat software pipeline across heads
    (scores of pair j+1 issued before AV/ones of pair j, crossing head
    boundaries) so the in-order PE queue hides the exp latency.
    Causal trapezoid: diagonal k-tiles stream only the valid q suffix
    (512/384/256/128); one [128,128] triangle mask.
  - Softmax denominators: ones-matmul row accumulated in PSUM,
    reciprocal on DVE, partition-broadcast via K=1 matmul.
  - O-projection of chunk qc-1 interleaved at head boundaries of
    attn(qc); fp16 partial outputs, tail stores spread over queues.
"""
import numpy as np

import concourse.bass as bass
import concourse.tile as tile
import concourse.mybir as mybir
from concourse import bacc
from concourse import bass_utils

F32 = mybir.dt.float32
F16 = mybir.dt.float16

DIM = 2048
S = 2048
B = 2
HL = 4           # q heads per core
FT = DIM // 128  # feature tiles
QC = 4           # q chunks (512) for attention
SCALE = 1.0 / np.sqrt(128.0)

_CACHE = {}


def _build():
    nc = bacc.Bacc("TRN2", target_bir_lowering=False, debug=False,
                   enable_asserts=True, num_devices=8)

    d_xt = nc.dram_tensor("xt_c", (DIM, S), F16, kind="ExternalInput").ap()
    d_wq = nc.dram_tensor("wq_c", (DIM, HL * 128), F16, kind="ExternalInput").ap()
    d_wk = nc.dram_tensor("wk_c", (DIM, 128), F16, kind="ExternalInput").ap()
    d_wv = nc.dram_tensor("wv_c", (DIM, 128), F16, kind="ExternalInput").ap()
    d_wo = nc.dram_tensor("wo_c", (HL * 128, DIM), F16, kind="ExternalInput").ap()
    d_cj = nc.dram_tensor("cjoin", (128, S), F16, kind="ExternalInput").ap()
    d_sj = nc.dram_tensor("sjoin", (128, S), F16, kind="ExternalInput").ap()
    d_mk = nc.dram_tensor("maskt", (128, 128), F16, kind="ExternalInput").ap()
    d_id = nc.dram_tensor("ident", (128, 128), F16, kind="ExternalInput").ap()
    d_ot = nc.dram_tensor("ot", (DIM, S), F16, kind="ExternalOutput").ap()

    Exp = mybir.ActivationFunctionType.Exp
    v_xt = d_xt.rearrange("(ft p) s -> p ft s", p=128)
    v_wq = d_wq.rearrange("(ft p) m -> p ft m", p=128)

    with tile.TileContext(nc) as tc:
        with tc.tile_pool(name="wts", bufs=1) as wp, \
             tc.tile_pool(name="acts", bufs=1) as ap:
            sb_warm = wp.tile([128, 128], F16)
            nc.vector.memset(sb_warm[:], 0.5)
            sb_id = wp.tile([128, 128], F16)
            sb_wq = wp.tile([128, FT, HL * 128], F16)
            sb_wk = wp.tile([128, FT, 128], F16)
            sb_wv = wp.tile([128, FT, 128], F16)
            sb_cj = wp.tile([128, S], F16)
            sb_sj = wp.tile([128, S], F16)
            sb_mk = wp.tile([128, 128], F16)
            sb_wo = wp.tile([128, HL, DIM], F16)
            ones16 = wp.tile([128, 1], F16)
            nc.vector.memset(ones16[:], 1.0)
            ones32 = wp.tile([1, 128], F32)
            nc.vector.memset(ones32[:], 1.0)

            sb_QT = ap.tile([128, HL, S], F16)
            sb_KT = ap.tile([128, S], F16)
            sb_V = ap.tile([128, S // 128, 128], F16)
            sb_oT = ap.tile([128, HL, S], F16)

            with tc.tile_pool(name="xT", bufs=2) as xT_p, \
                 tc.tile_pool(name="vt", bufs=2) as vt_p, \
                 tc.tile_pool(name="rope", bufs=2) as rp, \
                 tc.tile_pool(name="attn", bufs=4) as at_p, \
                 tc.tile_pool(name="rcp", bufs=2) as rc_p, \
                 tc.tile_pool(name="bcst", bufs=2) as bc_p, \
                 tc.tile_pool(name="otile", bufs=4) as ot_p, \
                 tc.tile_pool(name="P1", bufs=2, space="PSUM") as P1, \
                 tc.tile_pool(name="P2", bufs=2, space="PSUM") as P2, \
                 tc.tile_pool(name="P3", bufs=1, space="PSUM") as P3, \
                 tc.tile_pool(name="P4", bufs=1, space="PSUM") as P4:

                # PE warmup on a memset tile: lifts the HAM clock gate
                # while the DMA rings start up (~10us); no DMA deps.
                pwarm = P3.tile([128, 512], F32, tag="p3", name="pwarm")
                for _ in range(90):
                    nc.tensor.matmul(pwarm[:, 0:128], sb_warm[:], sb_warm[:],
                                     start=True, stop=True)

                def rope(T, c0):
                    # T: [128, 512] fp16 chunk at token offset c0
                    mc = rp.tile([128, 512], F16, tag="mc", name="mc")
                    ms = rp.tile([128, 512], F16, tag="ms", name="ms")
                    cjs = sb_cj[:, c0:c0 + 512]
                    sjs = sb_sj[:, c0:c0 + 512]
                    nc.gpsimd.tensor_mul(mc[:], T, cjs)
                    nc.vector.tensor_mul(ms[0:64, :], T[64:128, :], sjs[64:128, :])
                    nc.vector.tensor_mul(ms[64:128, :], T[0:64, :], sjs[0:64, :])
                    nc.vector.tensor_add(T, mc[:], ms[:])

                def chunk(sc):
                    s0 = sc * 512
                    xt = xT_p.tile([128, FT, 512], F16, name="xt")
                    if sc == 0:
                        nc.gpsimd.dma_start(sb_wk[:], d_wk.rearrange(
                            "(ft p) m -> p ft m", p=128))
                        nc.gpsimd.dma_start(sb_wv[:], d_wv.rearrange(
                            "(ft p) m -> p ft m", p=128))
                        for fi in range(FT):
                            nc.scalar.dma_start(sb_wq[:, fi, :], v_wq[:, fi, :])
                            eng = nc.sync if fi % 2 == 0 else nc.gpsimd
                            eng.dma_start(xt[:, fi, :], v_xt[:, fi, s0:s0 + 512])
                        nc.sync.dma_start(sb_id[:], d_id)
                        nc.gpsimd.dma_start(sb_cj[:], d_cj)
                        nc.gpsimd.dma_start(sb_sj[:], d_sj)
                        nc.gpsimd.dma_start(sb_mk[:], d_mk)
                        nc.gpsimd.dma_start(
                            sb_wo[:], d_wo.rearrange("(dv p) m -> p dv m", p=128))
                    else:
                        nc.sync.dma_start(xt[:], v_xt[:, :, s0:s0 + 512])

                    pa = P1.tile([128, 1024], F32, tag="p1", name="pa")
                    pb = P1.tile([128, 1024], F32, tag="p1", name="pb")
                    pc = P2.tile([128, 512], F32, tag="p2", name="pc")
                    pd = P2.tile([128, 512], F32, tag="p2", name="pd")
                    # pa: [V | K]; pb: [Q0 | Q1]; pc: Q2; pd: Q3
                    for fi in range(FT):
                        st, sp = (fi == 0), (fi == FT - 1)
                        nc.tensor.matmul(pa[:, 0:512], sb_wv[:, fi, :],
                                         xt[:, fi, :], start=st, stop=sp)
                        nc.tensor.matmul(pa[:, 512:1024], sb_wk[:, fi, :],
                                         xt[:, fi, :], start=st, stop=sp)
                        nc.tensor.matmul(pb[:, 0:512], sb_wq[:, fi, 0:128],
                                         xt[:, fi, :], start=st, stop=sp)
                        nc.tensor.matmul(pb[:, 512:1024], sb_wq[:, fi, 128:256],
                                         xt[:, fi, :], start=st, stop=sp)
                        nc.tensor.matmul(pc[:], sb_wq[:, fi, 256:384],
                                         xt[:, fi, :], start=st, stop=sp)
                        nc.tensor.matmul(pd[:], sb_wq[:, fi, 384:512],
                                         xt[:, fi, :], start=st, stop=sp)
                    vt = vt_p.tile([128, 512], F16, name="vt")
                    nc.scalar.copy(vt[:], pa[:, 0:512])
                    nc.scalar.copy(sb_KT[:, s0:s0 + 512], pa[:, 512:1024])
                    nc.scalar.copy(sb_QT[:, 0, s0:s0 + 512], pb[:, 0:512])
                    nc.scalar.copy(sb_QT[:, 1, s0:s0 + 512], pb[:, 512:1024])
                    nc.scalar.copy(sb_QT[:, 2, s0:s0 + 512], pc[:])
                    nc.scalar.copy(sb_QT[:, 3, s0:s0 + 512], pd[:])
                    ptv = P4.tile([128, 1024], F16, tag="p4", name="ptv")
                    for tl in range(4):
                        nc.tensor.transpose(
                            ptv[:, tl * 128:(tl + 1) * 128],
                            vt[:, tl * 128:(tl + 1) * 128], sb_id[:])
                    for tl in range(4):
                        nc.vector.tensor_copy(sb_V[:, sc * 4 + tl, :],
                                              ptv[:, tl * 128:(tl + 1) * 128])
                    rope(sb_KT[:, s0:s0 + 512], s0)
                    for h in range(HL):
                        rope(sb_QT[:, h, s0:s0 + 512], s0)

                def oproj_group(qc, oi, engine, st_eng):
                    pot = P2.tile([128, 512], F32, tag="p2", name="pot")
                    for dvi in range(HL):
                        nc.tensor.matmul(
                            pot[:], sb_wo[:, dvi, oi * 128:(oi + 1) * 128],
                            sb_oT[:, dvi, qc * 512:(qc + 1) * 512],
                            start=(dvi == 0), stop=(dvi == HL - 1))
                    otc = ot_p.tile([128, 512], F16, name="otc")
                    if engine == "v":
                        nc.vector.tensor_copy(otc[:], pot[:])
                    else:
                        nc.scalar.copy(otc[:], pot[:])
                    st_eng.dma_start(
                        d_ot[oi * 128:(oi + 1) * 128,
                             qc * 512:(qc + 1) * 512], otc[:])

                def attn_block(qc, op_src):
                    kp_n = (qc + 1) * 2
                    kmax = (qc + 1) * 4
                    q0 = qc * 512
                    po = {}
                    pon = {}

                    def width(ki):
                        jloc = ki - qc * 4
                        return 512 if jloc < 0 else 512 - jloc * 128

                    def emit_scores(h, kp):
                        psc = P1.tile([128, 1024], F32, tag="p1", name="psc")
                        at = at_p.tile([128, 1024], F16, tag="at", name="at")
                        for half in range(2):
                            ki = kp * 2 + half
                            qo = 512 - width(ki)
                            nc.tensor.matmul(
                                psc[:, half * 512 + qo:(half + 1) * 512],
                                sb_KT[:, ki * 128:(ki + 1) * 128],
                                sb_QT[:, h, q0 + qo:q0 + 512],
                                start=True, stop=True)
                        w0, w1 = width(kp * 2), width(kp * 2 + 1)
                        if w0 == 512 and w1 == 512:
                            nc.scalar.activation(at[:], psc[:], Exp, scale=SCALE)
                        else:
                            nc.scalar.activation(
                                at[:, 512 - w0:512], psc[:, 512 - w0:512],
                                Exp, scale=SCALE)
                            nc.scalar.activation(
                                at[:, 1024 - w1:1024], psc[:, 1024 - w1:1024],
                                Exp, scale=SCALE)
                        for half in range(2):
                            ki = kp * 2 + half
                            if ki >= qc * 4:
                                c0 = half * 512 + 512 - width(ki)
                                nc.vector.tensor_mul(
                                    at[:, c0:c0 + 128], at[:, c0:c0 + 128],
                                    sb_mk[:])
                        return at

                    def emit_av(h, kp, at):
                        if kp == 0:
                            # lazy allocation keeps the P2/P3 ring
                            # tenancy strictly sequential (after the
                            # previous head's pots/po are released)
                            po[h] = P3.tile([128, 512], F32, tag="p3",
                                            name="po")
                            pon[h] = P2.tile([128, 512], F32, tag="p2",
                                             name="pon")
                        for half in range(2):
                            ki = kp * 2 + half
                            qo = 512 - width(ki)
                            st, sp = (ki == 0), (ki == kmax - 1)
                            rhs = at[:, half * 512 + qo:half * 512 + 512]
                            nc.tensor.matmul(
                                po[h][:, qo:512] if qo else po[h][:],
                                sb_V[:, ki, :], rhs, start=st, stop=sp)
                            nc.tensor.matmul(
                                pon[h][0:1, qo:512] if qo else pon[h][0:1, :],
                                ones16[:], rhs, start=st, stop=sp)

                    def finish_head(h):
                        rc = rc_p.tile([1, 512], F32, name="rc")
                        nc.vector.reciprocal_approx_fast(rc[:], pon[h][0:1, :])
                        pbc = P1.tile([128, 1024], F32, tag="p1", name="pbc")
                        nc.tensor.matmul(pbc[:, 0:512], ones32[:], rc[:],
                                         start=True, stop=True)
                        bc = bc_p.tile([128, 512], F32, name="bc")
                        nc.vector.tensor_copy(bc[:], pbc[:, 0:512])
                        nc.vector.tensor_mul(
                            sb_oT[:, h, q0:q0 + 512], po[h][:], bc[:])
                        if op_src is not None:
                            for oi in range(h * 4, h * 4 + 4):
                                oproj_group(op_src, oi, "v", nc.sync)

                    prev = None
                    for h in range(HL):
                        for kp in range(kp_n):
                            at = emit_scores(h, kp)
                            if prev is not None:
                                emit_av(*prev)
                                if prev[1] == kp_n - 1:
                                    finish_head(prev[0])
                            prev = (h, kp, at)
                    emit_av(*prev)
                    finish_head(HL - 1)

                chunk(0)
                chunk(1)
                attn_block(0, None)
                chunk(2)
                attn_block(1, 0)
                chunk(3)
                attn_block(2, 1)
                attn_block(3, 2)
                # tail: O-proj of the last q-chunk; spread evac engines
                # and store queues
                for oi in range(FT):
                    oproj_group(QC - 1, oi, "v" if oi % 2 else "s",
                                nc.sync if oi % 2 else nc.gpsimd)

    nc.compile()
    return nc


def _prep_shards(x, freqs_cos, freqs_sin, wq, wk, wv, wo):
    perm = np.empty(128, dtype=np.int64)
    perm[0:64] = 2 * np.arange(64)
    perm[64:128] = 2 * np.arange(64) + 1

    cosT = np.ascontiguousarray(freqs_cos.T).astype(np.float32)
    sinT = np.ascontiguousarray(freqs_sin.T).astype(np.float32)
    cjoin = np.concatenate([cosT, cosT], axis=0).astype(np.float16)
    sjoin = np.concatenate([sinT, -sinT], axis=0).astype(np.float16)

    q_idx = np.arange(128)[None, :]
    k_idx = np.arange(128)[:, None]
    maskt = (q_idx >= k_idx).astype(np.float16)
    ident = np.eye(128, dtype=np.float16)

    xT = [np.ascontiguousarray(np.asarray(x[b]).T).astype(np.float16)
          for b in range(B)]

    in_maps = []
    for c in range(8):
        b, g = c // 4, c % 4
        wq_g = np.ascontiguousarray(
            wq[:, g * 512:(g + 1) * 512].reshape(DIM, 4, 128)[:, :, perm]
            .reshape(DIM, 512)).astype(np.float16)
        wk_g = np.ascontiguousarray(
            wk[:, g * 128:(g + 1) * 128][:, perm]).astype(np.float16)
        wv_g = np.ascontiguousarray(
            wv[:, g * 128:(g + 1) * 128]).astype(np.float16)
        wo_g = np.ascontiguousarray(
            wo[g * 512:(g + 1) * 512, :]).astype(np.float16)
        in_maps.append({
            "xt_c": xT[b],
            "wq_c": wq_g, "wk_c": wk_g, "wv_c": wv_g, "wo_c": wo_g,
            "cjoin": cjoin, "sjoin": sjoin, "maskt": maskt, "ident": ident,
        })
    return in_maps


def _assemble(results):
    out = np.zeros((B, S, DIM), dtype=np.float32)
    for c in range(8):
        out[c // 4] += results[c]["ot"].T.astype(np.float32)
    return out


def kernel(x, freqs_cos, freqs_sin, wq, wk, wv, wo):
    x = np.asarray(x, dtype=np.float32)
    if "nc" not in _CACHE:
        _CACHE["nc"] = _build()
    nc = _CACHE["nc"]
    in_maps = _prep_shards(x, np.asarray(freqs_cos), np.asarray(freqs_sin),
                           np.asarray(wq), np.asarray(wk), np.asarray(wv),
                           np.asarray(wo))
    res = bass_utils.run_bass_kernel_spmd(nc, in_maps, core_ids=list(range(8)))
    return _assemble(res.results)


# revision 15
# speedup vs baseline: 1.0364x; 1.0364x over previous
"""Tensor-parallel GQA attention kernel for 8 Trainium2 NeuronCores.

Problem: x[2,2048,2048] -> Attention(16 q heads, 4 kv heads, rotary,
causal) -> out[2,2048,2048].

Sharding: core c handles batch b=c//4 and kv-group g=c%4 (4 q-heads +
1 kv-head). Each core computes its heads' attention output and a
partial O-projection [DIM, S] (output-dim major, fp16); the host sums
the 4 partials per batch and transposes.

v4 design (pure fp16 matmuls, fp32 PSUM):
  - x pre-transposed on host (xT [DIM,S] fp16); contiguous DMA loads,
    chunk-0 split per feature-tile across queues in consumption order.
    Warmup matmuls on a memset tile bridge the ~10us DMA-ring startup
    and lift the HAM clock gate.
  - Projections fi-outer per 512-token chunk with 6 concurrently open
    PSUM groups (V,K,Q0..Q3); V transposed via PE into a dedicated
    fp16 PSUM bank (4 transposes, then 4 evac copies - no same-bank
    PE-write/DVE-read interleave).
  - Attention blocks interleave between projection chunks (attn(qc)
    right after chunk qc+1) so cross-phase queue backlogs hide under
    matmul streams. One unified 8-bank PSUM map serves both phases:
    P1 2x[128,1024]f32 (chunk V|K + Q0|Q1 / attn score pairs + bcast),
    P2 2x[128,512]f32 (chunk Q2,Q3 / attn ones-rows + O-proj tiles),
    P3 1x[128,512]f32 (warmup / attn AV accumulator),
    P4 1x[128,1024]f16 (V transposes).
  - Attention per 512-q chunk: flat software pipeline across heads
    (scores of pair j+1 issued before AV/ones of pair j, crossing head
    boundaries) so the in-order PE queue hides the exp latency.
    Causal trapezoid: diagonal k-tiles stream only the valid q suffix
    (512/384/256/128); one [128,128] triangle mask.
  - Softmax denominators: ones-matmul row accumulated in PSUM,
    reciprocal on DVE, partition-broadcast via K=1 matmul.
  - O-projection of chunk qc-1 interleaved at head boundaries of
    attn(qc); fp16 partial outputs, tail stores spread over queues.
"""
import numpy as np

import concourse.bass as bass
import concourse.tile as tile
import concourse.mybir as mybir
from concourse import bacc
from concourse import bass_utils

F32 = mybir.dt.float32
F16 = mybir.dt.float16

DIM = 2048
S = 2048
B = 2
HL = 4           # q heads per core
FT = DIM // 128  # feature tiles
QC = 4           # q chunks (512) for attention
SCALE = 1.0 / np.sqrt(128.0)

_CACHE = {}


def _build():
    nc = bacc.Bacc("TRN2", target_bir_lowering=False, debug=False,
                   enable_asserts=True, num_devices=8)

    d_xt = nc.dram_tensor("xt_c", (DIM, S), F16, kind="ExternalInput").ap()
    d_wq = nc.dram_tensor("wq_c", (DIM, HL * 128), F16, kind="ExternalInput").ap()
    d_wk = nc.dram_tensor("wk_c", (DIM, 128), F16, kind="ExternalInput").ap()
    d_wv = nc.dram_tensor("wv_c", (DIM, 128), F16, kind="ExternalInput").ap()
    d_wo = nc.dram_tensor("wo_c", (HL * 128, DIM), F16, kind="ExternalInput").ap()
    d_cj = nc.dram_tensor("cjoin", (128, S), F16, kind="ExternalInput").ap()
    d_sj = nc.dram_tensor("sjoin", (128, S), F16, kind="ExternalInput").ap()
    d_mk = nc.dram_tensor("maskt", (128, 128), F16, kind="ExternalInput").ap()
    d_id = nc.dram_tensor("ident", (128, 128), F16, kind="ExternalInput").ap()
    d_ot = nc.dram_tensor("ot", (DIM, S), F16, kind="ExternalOutput").ap()

    Exp = mybir.ActivationFunctionType.Exp
    v_xt = d_xt.rearrange("(ft p) s -> p ft s", p=128)
    v_wq = d_wq.rearrange("(ft p) m -> p ft m", p=128)

    with tile.TileContext(nc) as tc:
        with tc.tile_pool(name="wts", bufs=1) as wp, \
             tc.tile_pool(name="acts", bufs=1) as ap:
            sb_warm = wp.tile([128, 128], F16)
            nc.vector.memset(sb_warm[:], 0.5)
            sb_id = wp.tile([128, 128], F16)
            sb_wq = wp.tile([128, FT, HL * 128], F16)
            sb_wk = wp.tile([128, FT, 128], F16)
            sb_wv = wp.tile([128, FT, 128], F16)
            sb_cj = wp.tile([128, S], F16)
            sb_sj = wp.tile([128, S], F16)
            sb_mk = wp.tile([128, 128], F16)
            sb_wo = wp.tile([128, HL, DIM], F16)
            ones16 = wp.tile([128, 1], F16)
            nc.vector.memset(ones16[:], 1.0)
            ones32 = wp.tile([1, 128], F32)
            nc.vector.memset(ones32[:], 1.0)

            sb_QT = ap.tile([128, HL, S], F16)
            sb_KT = ap.tile([128, S], F16)
            sb_V = ap.tile([128, S // 128, 128], F16)
            sb_oT = ap.tile([128, HL, S], F16)

            with tc.tile_pool(name="xT", bufs=2) as xT_p, \
                 tc.tile_pool(name="vt", bufs=2) as vt_p, \
                 tc.tile_pool(name="rope", bufs=2) as rp, \
                 tc.tile_pool(name="attn", bufs=4) as at_p, \
                 tc.tile_pool(name="rcp", bufs=2) as rc_p, \
                 tc.tile_pool(name="bcst", bufs=2) as bc_p, \
                 tc.tile_pool(name="otile", bufs=4) as ot_p, \
                 tc.tile_pool(name="P1", bufs=2, space="PSUM") as P1, \
                 tc.tile_pool(name="P2", bufs=2, space="PSUM") as P2, \
                 tc.tile_pool(name="P3", bufs=1, space="PSUM") as P3, \
                 tc.tile_pool(name="P4", bufs=1, space="PSUM") as P4:

                # PE warmup on a memset tile: lifts the HAM clock gate
                # while the DMA rings start up (~10us); no DMA deps.
                pwarm = P3.tile([128, 512], F32, tag="p3", name="pwarm")
                for _ in range(90):
                    nc.tensor.matmul(pwarm[:, 0:128], sb_warm[:], sb_warm[:],
                                     start=True, stop=True)

                def rope(T, c0):
                    # T: [128, 512] fp16 chunk at token offset c0
                    mc = rp.tile([128, 512], F16, tag="mc", name="mc")
                    ms = rp.tile([128, 512], F16, tag="ms", name="ms")
                    cjs = sb_cj[:, c0:c0 + 512]
                    sjs = sb_sj[:, c0:c0 + 512]
                    nc.gpsimd.tensor_mul(mc[:], T, cjs)
                    nc.vector.tensor_mul(ms[0:64, :], T[64:128, :], sjs[64:128, :])
                    nc.vector.tensor_mul(ms[64:128, :], T[0:64, :], sjs[0:64, :])
                    nc.vector.tensor_add(T, mc[:], ms[:])

                def chunk(sc):
                    s0 = sc * 512
                    xt = xT_p.tile([128, FT, 512], F16, name="xt")
                    if sc == 0:
                        nc.gpsimd.dma_start(sb_wk[:], d_wk.rearrange(
                            "(ft p) m -> p ft m", p=128))
                        nc.gpsimd.dma_start(sb_wv[:], d_wv.rearrange(
                            "(ft p) m -> p ft m", p=128))
                        for fi in range(FT):
                            nc.scalar.dma_start(sb_wq[:, fi, :], v_wq[:, fi, :])
                            eng = nc.sync if fi % 2 == 0 else nc.gpsimd
                            eng.dma_start(xt[:, fi, :], v_xt[:, fi, s0:s0 + 512])
                        nc.sync.dma_start(sb_id[:], d_id)
                        nc.sync.dma_start(sb_cj[:], d_cj)
                        nc.sync.dma_start(sb_sj[:], d_sj)
                        nc.sync.dma_start(sb_mk[:], d_mk)
                        nc.sync.dma_start(
                            sb_wo[:], d_wo.rearrange("(dv p) m -> p dv m", p=128))
                    else:
                        nc.sync.dma_start(xt[:], v_xt[:, :, s0:s0 + 512])

                    pa = P1.tile([128, 1024], F32, tag="p1", name="pa")
                    pb = P1.tile([128, 1024], F32, tag="p1", name="pb")
                    pc = P2.tile([128, 512], F32, tag="p2", name="pc")
                    pd = P2.tile([128, 512], F32, tag="p2", name="pd")
                    # pa: [V | K]; pb: [Q0 | Q1]; pc: Q2; pd: Q3
                    for fi in range(FT):
                        st, sp = (fi == 0), (fi == FT - 1)
                        nc.tensor.matmul(pa[:, 0:512], sb_wv[:, fi, :],
                                         xt[:, fi, :], start=st, stop=sp)
                        nc.tensor.matmul(pa[:, 512:1024], sb_wk[:, fi, :],
                                         xt[:, fi, :], start=st, stop=sp)
                        nc.tensor.matmul(pb[:, 0:512], sb_wq[:, fi, 0:128],
                                         xt[:, fi, :], start=st, stop=sp)
                        nc.tensor.matmul(pb[:, 512:1024], sb_wq[:, fi, 128:256],
                                         xt[:, fi, :], start=st, stop=sp)
                        nc.tensor.matmul(pc[:], sb_wq[:, fi, 256:384],
                                         xt[:, fi, :], start=st, stop=sp)
                        nc.tensor.matmul(pd[:], sb_wq[:, fi, 384:512],
                                         xt[:, fi, :], start=st, stop=sp)
                    # evacuations split across ScalarE/DVE so the next
                    # consumer of each PSUM bank unblocks quickly
                    vt = vt_p.tile([128, 512], F16, name="vt")
                    nc.scalar.copy(vt[:], pa[:, 0:512])
                    nc.vector.tensor_copy(sb_KT[:, s0:s0 + 512], pa[:, 512:1024])
                    nc.scalar.copy(sb_QT[:, 0, s0:s0 + 512], pb[:, 0:512])
                    nc.vector.tensor_copy(sb_QT[:, 1, s0:s0 + 512],
                                          pb[:, 512:1024])
                    nc.scalar.copy(sb_QT[:, 2, s0:s0 + 512], pc[:])
                    nc.vector.tensor_copy(sb_QT[:, 3, s0:s0 + 512], pd[:])
                    ptv = P4.tile([128, 1024], F16, tag="p4", name="ptv")
                    for tl in range(4):
                        nc.tensor.transpose(
                            ptv[:, tl * 128:(tl + 1) * 128],
                            vt[:, tl * 128:(tl + 1) * 128], sb_id[:])

                    def vcopies():
                        for tl in range(4):
                            nc.vector.tensor_copy(
                                sb_V[:, sc * 4 + tl, :],
                                ptv[:, tl * 128:(tl + 1) * 128])

                    rope_fns = [lambda: rope(sb_KT[:, s0:s0 + 512], s0)]
                    for h in range(HL):
                        rope_fns.append(
                            lambda h=h: rope(sb_QT[:, h, s0:s0 + 512], s0))
                    return vcopies, rope_fns

                def oproj_group(qc, oi, engine, st_eng):
                    pot = P2.tile([128, 512], F32, tag="p2", name="pot")
                    for dvi in range(HL):
                        nc.tensor.matmul(
                            pot[:], sb_wo[:, dvi, oi * 128:(oi + 1) * 128],
                            sb_oT[:, dvi, qc * 512:(qc + 1) * 512],
                            start=(dvi == 0), stop=(dvi == HL - 1))
                    otc = ot_p.tile([128, 512], F16, name="otc")
                    if engine == "v":
                        nc.vector.tensor_copy(otc[:], pot[:])
                    else:
                        nc.scalar.copy(otc[:], pot[:])
                    st_eng.dma_start(
                        d_ot[oi * 128:(oi + 1) * 128,
                             qc * 512:(qc + 1) * 512], otc[:])

                def attn_block(qc, op_src, defer=None):
                    kp_n = (qc + 1) * 2
                    kmax = (qc + 1) * 4
                    q0 = qc * 512
                    po = {}
                    pon = {}

                    def width(ki):
                        jloc = ki - qc * 4
                        return 512 if jloc < 0 else 512 - jloc * 128

                    def emit_scores(h, kp):
                        psc = P1.tile([128, 1024], F32, tag="p1", name="psc")
                        at = at_p.tile([128, 1024], F16, tag="at", name="at")
                        for half in range(2):
                            ki = kp * 2 + half
                            qo = 512 - width(ki)
                            nc.tensor.matmul(
                                psc[:, half * 512 + qo:(half + 1) * 512],
                                sb_KT[:, ki * 128:(ki + 1) * 128],
                                sb_QT[:, h, q0 + qo:q0 + 512],
                                start=True, stop=True)
                        w0, w1 = width(kp * 2), width(kp * 2 + 1)
                        if w0 == 512 and w1 == 512:
                            nc.scalar.activation(at[:], psc[:], Exp, scale=SCALE)
                        else:
                            nc.scalar.activation(
                                at[:, 512 - w0:512], psc[:, 512 - w0:512],
                                Exp, scale=SCALE)
                            nc.scalar.activation(
                                at[:, 1024 - w1:1024], psc[:, 1024 - w1:1024],
                                Exp, scale=SCALE)
                        for half in range(2):
                            ki = kp * 2 + half
                            if ki >= qc * 4:
                                c0 = half * 512 + 512 - width(ki)
                                nc.vector.tensor_mul(
                                    at[:, c0:c0 + 128], at[:, c0:c0 + 128],
                                    sb_mk[:])
                        return at

                    def emit_av(h, kp, at):
                        if kp == 0:
                            # lazy allocation keeps the P2/P3 ring
                            # tenancy strictly sequential (after the
                            # previous head's pots/po are released)
                            po[h] = P3.tile([128, 512], F32, tag="p3",
                                            name="po")
                            pon[h] = P2.tile([128, 512], F32, tag="p2",
                                             name="pon")
                        for half in range(2):
                            ki = kp * 2 + half
                            qo = 512 - width(ki)
                            st, sp = (ki == 0), (ki == kmax - 1)
                            rhs = at[:, half * 512 + qo:half * 512 + 512]
                            nc.tensor.matmul(
                                po[h][:, qo:512] if qo else po[h][:],
                                sb_V[:, ki, :], rhs, start=st, stop=sp)
                            nc.tensor.matmul(
                                pon[h][0:1, qo:512] if qo else pon[h][0:1, :],
                                ones16[:], rhs, start=st, stop=sp)

                    def finish_head(h):
                        rc = rc_p.tile([1, 512], F32, name="rc")
                        nc.vector.reciprocal_approx_fast(rc[:], pon[h][0:1, :])
                        pbc = P1.tile([128, 1024], F32, tag="p1", name="pbc")
                        nc.tensor.matmul(pbc[:, 0:512], ones32[:], rc[:],
                                         start=True, stop=True)
                        bc = bc_p.tile([128, 512], F32, name="bc")
                        nc.vector.tensor_copy(bc[:], pbc[:, 0:512])
                        nc.vector.tensor_mul(
                            sb_oT[:, h, q0:q0 + 512], po[h][:], bc[:])
                        if defer is not None:
                            for fn in defer[h]:
                                fn()
                        if op_src is not None:
                            for oi in range(h * 4, h * 4 + 4):
                                oproj_group(op_src, oi, "v", nc.sync)

                    prev = None
                    for h in range(HL):
                        for kp in range(kp_n):
                            at = emit_scores(h, kp)
                            if prev is not None:
                                emit_av(*prev)
                                if prev[1] == kp_n - 1:
                                    finish_head(prev[0])
                            prev = (h, kp, at)
                    emit_av(*prev)
                    finish_head(HL - 1)

                for sc in range(3):
                    vc, rf = chunk(sc)
                    vc()
                    for fn in rf:
                        fn()
                vc3, rf3 = chunk(3)
                # chunk 3's V copies and ropes are only needed by
                # attn3; defer them into attn0's head boundaries so
                # the phase-transition queues stay clear
                attn_block(0, None, defer=[
                    [vc3, rf3[0]], [rf3[1]], [rf3[2], rf3[3]], [rf3[4]]])
                attn_block(1, 0)
                attn_block(2, 1)
                attn_block(3, 2)
                # tail: O-proj of the last q-chunk; spread evac engines
                # and store queues
                for oi in range(FT):
                    oproj_group(QC - 1, oi, "v" if oi % 2 else "s",
                                nc.sync if oi % 2 else nc.gpsimd)

    nc.compile()
    return nc


def _prep_shards(x, freqs_cos, freqs_sin, wq, wk, wv, wo):
    perm = np.empty(128, dtype=np.int64)
    perm[0:64] = 2 * np.arange(64)
    perm[64:128] = 2 * np.arange(64) + 1

    cosT = np.ascontiguousarray(freqs_cos.T).astype(np.float32)
    sinT = np.ascontiguousarray(freqs_sin.T).astype(np.float32)
    cjoin = np.concatenate([cosT, cosT], axis=0).astype(np.float16)
    sjoin = np.concatenate([sinT, -sinT], axis=0).astype(np.float16)

    q_idx = np.arange(128)[None, :]
    k_idx = np.arange(128)[:, None]
    maskt = (q_idx >= k_idx).astype(np.float16)
    ident = np.eye(128, dtype=np.float16)

    xT = [np.ascontiguousarray(np.asarray(x[b]).T).astype(np.float16)
          for b in range(B)]

    in_maps = []
    for c in range(8):
        b, g = c // 4, c % 4
        wq_g = np.ascontiguousarray(
            wq[:, g * 512:(g + 1) * 512].reshape(DIM, 4, 128)[:, :, perm]
            .reshape(DIM, 512)).astype(np.float16)
        wk_g = np.ascontiguousarray(
            wk[:, g * 128:(g + 1) * 128][:, perm]).astype(np.float16)
        wv_g = np.ascontiguousarray(
            wv[:, g * 128:(g + 1) * 128]).astype(np.float16)
        wo_g = np.ascontiguousarray(
            wo[g * 512:(g + 1) * 512, :]).astype(np.float16)
        in_maps.append({
            "xt_c": xT[b],
            "wq_c": wq_g, "wk_c": wk_g, "wv_c": wv_g, "wo_c": wo_g,
            "cjoin": cjoin, "sjoin": sjoin, "maskt": maskt, "ident": ident,
        })
    return in_maps


def _assemble(results):
    out = np.zeros((B, S, DIM), dtype=np.float32)
    for c in range(8):
        out[c // 4] += results[c]["ot"].T.astype(np.float32)
    return out


def kernel(x, freqs_cos, freqs_sin, wq, wk, wv, wo):
    x = np.asarray(x, dtype=np.float32)
    if "nc" not in _CACHE:
        _CACHE["nc"] = _build()
    nc = _CACHE["nc"]
    in_maps = _prep_shards(x, np.asarray(freqs_cos), np.asarray(freqs_sin),
                           np.asarray(wq), np.asarray(wk), np.asarray(wv),
                           np.asarray(wo))
    res = bass_utils.run_bass_kernel_spmd(nc, in_maps, core_ids=list(range(8)))
    return _assemble(res.results)
